# revision 10
# baseline (speedup 1.0000x reference)
"""Trainium2 Bass kernel for nn_GravityLayer (gnn_message_passing).

Sharding: causal fold — core c owns rows [64c,64c+64) ("lo") and
[1024-64(c+1),1024-64c) ("hi"). Per-core j-work is uniform: Jlo+Jhi = 1088
= 17 j-tiles of 64. All per-core variation is baked into host-prepared
input *content* (cat-j gathers, masks, one-hot placement), so the SPMD
program is identical on all 8 cores.

Device pipeline per (batch, j-tile): dist via K=50 Gram matmul
(-2 pos_i.pos_j + ri + rj), clamp+sqrt(ACT)+recip*mask(DVE); pair-MLP as
block-diagonal 2-pair fp32r matmuls [6,96]->gelu->[96,96]->gelu->[96,8]
(4 shifted W3 variants accumulate into one [8,512] PSUM bank); reshape to
G[64i,64j] (2 DMAs); G=(G+b3)*R; PE-transpose; contraction with
[pos_j|1] accumulates [64,49]; one-hot perm matmul places into the
fold-row frame [128, 49] PSUM accumulator. Finals elementwise on DVE.
"""

import sys
import numpy as np

for _p in ("/opt/trn_rl_repo",):
    if _p not in sys.path:
        sys.path.insert(0, _p)

B, N, D = 2, 1024, 48
NCORES = 8
NB = 64           # fold block / j-tile size
NJT = 17          # j-tiles per batch per core
CAT = NJT * NB    # 1088
EPS2 = 1e-4       # 0.01**2

_BUILD_CACHE = {}


def _build(dampingf, dtf, mb2f, fb3f):
    import concourse.bass as bass
    import concourse.bacc as bacc
    import concourse.tile as tile
    from concourse import mybir
    from contextlib import ExitStack

    f32 = mybir.dt.float32
    f32r = mybir.dt.float32r
    AF = mybir.ActivationFunctionType
    ALU = mybir.AluOpType
    AP = bass.AP

    nc = bacc.Bacc("TRN2")

    def inp(name, shape, dt_=None):
        return nc.declare_dram_parameter(name, list(shape), dt_ or f32,
                                         isOutput=False)

    piT_gram = inp("piT_gram", (B, 50, CAT))
    piT_mlp = inp("piT_mlp", (B, 48, CAT))
    pjT = inp("pjT", (B, 50, CAT))
    pj1 = inp("pj1", (B, CAT, 49))
    perm = inp("perm", (NJT, NB, 128))
    maskc = inp("maskc", (NJT, NB, NB))
    pos_nat = inp("pos_nat", (B, 128, D))
    vel_nat = inp("vel_nat", (B, 128, D))
    m_w1 = inp("m_w1", (48, 12))
    m_b1c = inp("m_b1c", (12, 1))
    m_w2 = inp("m_w2", (12, 1))
    w1blk_i = inp("w1blk", (6, 96), f32r)
    w2blk_i = inp("w2blk", (96, 96), f32r)
    w3s_i = inp("w3s", (96, 4, 8), f32r)
    b1blk_i = inp("b1blk", (96, 1))
    b2blk_i = inp("b2blk", (96, 1))
    ident_i = inp("ident64", (64, 64))

    out_pos = nc.declare_dram_parameter("out_pos", [B, 128, D], f32, isOutput=True)
    out_vel = nc.declare_dram_parameter("out_vel", [B, 128, D], f32, isOutput=True)

    with ExitStack() as ctx:
        tc = ctx.enter_context(tile.TileContext(nc))
        S = ctx.enter_context(tc.tile_pool(name="S", bufs=1))
        Xp = ctx.enter_context(tc.tile_pool(name="Xp", bufs=3))
        Hp = ctx.enter_context(tc.tile_pool(name="Hp", bufs=2))
        Gp = ctx.enter_context(tc.tile_pool(name="Gp", bufs=2))
        PZ = ctx.enter_context(tc.tile_pool(name="PZ", bufs=1, space="PSUM"))
        PZ2 = ctx.enter_context(tc.tile_pool(name="PZ2", bufs=1, space="PSUM"))
        PF = ctx.enter_context(tc.tile_pool(name="PF", bufs=1, space="PSUM"))
        PG = ctx.enter_context(tc.tile_pool(name="PG", bufs=1, space="PSUM"))
        PM = ctx.enter_context(tc.tile_pool(name="PM", bufs=1, space="PSUM"))

        dma = nc.sync.dma_start

        # ---- resident SBUF ----
        sb_pjT = S.tile([50, B, CAT], f32)
        dma(out=sb_pjT[:], in_=pjT.rearrange("b r c -> r b c"))
        sb_pig = S.tile([50, B, CAT], f32)
        dma(out=sb_pig[:], in_=piT_gram.rearrange("b r c -> r b c"))
        sb_pim = S.tile([48, B, CAT], f32)
        dma(out=sb_pim[:], in_=piT_mlp.rearrange("b r c -> r b c"))
        sb_pj1 = S.tile([NB, B, NJT, 49], f32)
        for b in range(B):
            dma(out=sb_pj1[:, b], in_=pj1[b].rearrange("(t j) c -> j t c", j=NB))
        sb_perm = S.tile([NB, NJT, 128], f32)
        dma(out=sb_perm[:], in_=perm.rearrange("t j c -> j t c"))
        sb_mask = S.tile([NB, NJT, NB], f32)
        dma(out=sb_mask[:], in_=maskc.rearrange("t j c -> j t c"))
        sb_pos = S.tile([128, B, D], f32)
        dma(out=sb_pos[:], in_=pos_nat.rearrange("b i c -> i b c"))
        sb_vel = S.tile([128, B, D], f32)
        dma(out=sb_vel[:], in_=vel_nat.rearrange("b i c -> i b c"))
        sb_mw1 = S.tile([48, 12], f32)
        dma(out=sb_mw1[:], in_=m_w1[:])
        sb_mb1 = S.tile([12, 1], f32)
        dma(out=sb_mb1[:], in_=m_b1c[:])
        sb_mw2 = S.tile([12, 1], f32)
        dma(out=sb_mw2[:], in_=m_w2[:])
        w1blk = S.tile([6, 96], f32r)
        dma(out=w1blk[:], in_=w1blk_i[:])
        w2blk = S.tile([96, 96], f32r)
        dma(out=w2blk[:], in_=w2blk_i[:])
        w3s = S.tile([96, 4, 8], f32r)
        dma(out=w3s[:], in_=w3s_i[:])
        b1blk = S.tile([96, 1], f32)
        dma(out=b1blk[:], in_=b1blk_i[:])
        b2blk = S.tile([96, 1], f32)
        dma(out=b2blk[:], in_=b2blk_i[:])
        ident = S.tile([64, 64], f32)
        dma(out=ident[:], in_=ident_i[:])

        sb_mb2 = S.tile([1, 1], f32)
        nc.vector.memset(sb_mb2[:], float(mb2f))
        ones11 = S.tile([1, 1], f32)
        nc.vector.memset(ones11[:], 1.0)
        ones164 = S.tile([1, 64], f32)
        nc.vector.memset(ones164[:], 1.0)
        zeros64 = S.tile([64, 64], f32)
        nc.vector.memset(zeros64[:], 0.0)

        D_all = S.tile([NB, B, NJT, NB], f32)
        R_all = S.tile([NB, B, NJT, NB], f32)
        m_i = S.tile([1, B, CAT], f32)
        m_j = S.tile([1, B, CAT], f32)

        SEGS = [(0, 512), (512, 512), (1024, 64)]

        # ---- phase B: masses (gelu, then softplus) ----
        mh_tiles = {}
        for b in range(B):
            for tag, src in (("i", sb_pim[:, b]), ("j", sb_pjT[0:48, b])):
                mh = Hp.tile([12, CAT], f32, tag="mh" + tag + str(b))
                for o, w in SEGS:
                    pm1 = PZ.tile([12, 512], f32, tag="z1")
                    nc.tensor.matmul(
                        pm1[:, 0:w],
                        sb_mw1[:],
                        src[:, o : o + w],
                    )
                    nc.scalar.activation(
                        mh[:, o : o + w], pm1[:, 0:w], AF.Gelu, bias=sb_mb1[:]
                    )
                mh_tiles[(b, tag)] = mh
        tc.no_sync_barrier()
        # softplus(x) = ln(1 + exp(x)) — exp and ln live in one table set
        for b in range(B):
            for tag, dst in (("i", m_i[:, b]), ("j", m_j[:, b])):
                mh = mh_tiles[(b, tag)]
                for o, w in SEGS:
                    pm2 = PF.tile([1, 512], f32, tag="F")
                    nc.tensor.matmul(
                        pm2[:, 0:w],
                        sb_mw2[:],
                        mh[:, o : o + w],
                    )
                    spt = Hp.tile([1, 512], f32, tag="sp")
                    nc.scalar.activation(
                        spt[:, 0:w], pm2[:, 0:w], AF.Exp, bias=sb_mb2[:]
                    )
                    nc.scalar.activation(
                        dst[:, o : o + w], spt[:, 0:w], AF.Ln, bias=1.0
                    )

        tc.no_sync_barrier()
        # ---- phase A: distances ----
        for b in range(B):
            for jt in range(NJT):
                c0 = jt * NB
                pd2 = PG.tile([64, 64], f32, tag="gtp")
                nc.tensor.matmul(
                    pd2[:],
                    sb_pig[:, b, c0 : c0 + NB],
                    sb_pjT[:, b, c0 : c0 + NB],
                )
                nc.vector.tensor_scalar_max(pd2[:], pd2[:], EPS2)
                nc.scalar.activation(D_all[:, b, jt], pd2[:], AF.Sqrt)
                nc.vector.reciprocal(R_all[:, b, jt], D_all[:, b, jt])
                nc.vector.tensor_mul(R_all[:, b, jt], R_all[:, b, jt], sb_mask[:, jt])

        tc.no_sync_barrier()
        # ---- phase C: force MLP ----
        FM = PM.tile([128, B * 49], f32)
        for b in range(B):
            for jt in range(NJT):
                c0 = jt * NB
                # broadcast tiles for m_i (per-row) and m_j (per-col)
                micol = PG.tile([64, 1], f32, tag="fm")
                nc.tensor.matmul(
                    micol[:], m_i[:, b, c0 : c0 + NB], ones11[:]
                )
                MIB = Gp.tile([64, 64], f32, tag="MIB")
                nc.vector.tensor_scalar_add(MIB[:], zeros64[:], micol[:])
                mjb = PG.tile([64, 64], f32, tag="gtp")
                nc.tensor.matmul(mjb[:], ones164[:], m_j[:, b, c0 : c0 + NB])
                MJB = Gp.tile([64, 64], f32, tag="MJB")
                nc.vector.tensor_copy(MJB[:], mjb[:])

                X = Xp.tile([6, 2048], f32r, tag="X")
                xap = X[:]
                # rows {0,3}: dist, i-major flatten of the [64,64] D tile
                nc.gpsimd.dma_start(
                    out=AP(tensor=xap.tensor, offset=xap.offset,
                           ap=[[3 * 2048, 2], [1, 2048]]),
                    in_=D_all[:, b, jt].bitcast(f32r),
                )
                # rows {1,4}: m_i broadcast tile, i-major flatten
                nc.gpsimd.dma_start(
                    out=AP(tensor=xap.tensor, offset=xap.offset + 2048,
                           ap=[[3 * 2048, 2], [1, 2048]]),
                    in_=MIB[:].bitcast(f32r),
                )
                # rows {2,5}: m_j broadcast tile
                nc.gpsimd.dma_start(
                    out=AP(tensor=xap.tensor, offset=xap.offset + 2 * 2048,
                           ap=[[3 * 2048, 2], [1, 2048]]),
                    in_=MJB[:].bitcast(f32r),
                )

                F = PF.tile([8, 512], f32, tag="F")
                for p in range(2):
                    z1 = PZ.tile([96, 1024], f32, tag="z1")
                    for q in range(2):
                        o = 512 * q
                        nc.tensor.matmul(
                            z1[:, o : o + 512],
                            w1blk[:],
                            X[:, 1024 * p + o : 1024 * p + o + 512],
                        )
                    h1 = Hp.tile([96, 1024], f32r, tag="h1")
                    nc.scalar.activation(h1[:], z1[:], AF.Gelu, bias=b1blk[:])
                    z2 = PZ2.tile([96, 1024], f32, tag="z2")
                    for q in range(2):
                        o = 512 * q
                        nc.tensor.matmul(
                            z2[:, o : o + 512],
                            w2blk[:],
                            h1[:, o : o + 512],
                        )
                    h2 = Hp.tile([96, 1024], f32r, tag="h2")
                    nc.scalar.activation(h2[:], z2[:], AF.Gelu, bias=b2blk[:])
                    for q in range(2):
                        s = 2 * p + q
                        nc.tensor.matmul(
                            F[:],
                            w3s[:, s],
                            h2[:, 512 * q : 512 * q + 512],
                            start=(s == 0),
                            stop=(s == 3),
                        )
                f8 = Gp.tile([8, 512], f32, tag="f8")
                nc.vector.tensor_copy(f8[:], F[:])
                G = Gp.tile([64, 64], f32, tag="G")
                f8ap = f8[:]
                for a in range(2):
                    dma(
                        out=G[32 * a : 32 * a + 32, :],
                        in_=AP(tensor=f8ap.tensor, offset=f8ap.offset + a * 512,
                               ap=[[2 * 512, 4], [1, 512]]),
                    )
                nc.vector.scalar_tensor_tensor(
                    G[:], G[:], fb3f, R_all[:, b, jt], ALU.add, ALU.mult
                )
                gtp = PG.tile([64, 64], f32, tag="gtp")
                nc.tensor.transpose(gtp[:], G[:], ident[:])
                gT = Gp.tile([64, 64], f32, tag="gT")
                nc.vector.tensor_copy(gT[:], gtp[:])
                fm = PG.tile([64, 49], f32, tag="fm")
                nc.tensor.matmul(fm[:], gT[:], sb_pj1[:, b, jt])
                fm_sb = Gp.tile([64, 49], f32, tag="fmsb")
                nc.vector.tensor_copy(fm_sb[:], fm[:])
                nc.tensor.matmul(
                    FM[:, 49 * b : 49 * b + 49],
                    sb_perm[:, jt],
                    fm_sb[:],
                    start=(jt == 0),
                    stop=(jt == NJT - 1),
                )

        # ---- finals ----
        for b in range(B):
            fm_b = FM[:, 49 * b : 49 * b + 49]
            mcol = PG.tile([128, 1], f32, tag="fm")
            nc.tensor.matmul(mcol[0:64], m_i[:, b, 0:64], ones11[:])
            nc.tensor.matmul(
                mcol[64:128], m_i[:, b, 16 * NB : 16 * NB + 64], ones11[:]
            )
            mre = Gp.tile([128, 1], f32, tag="mre")
            nc.vector.tensor_scalar_add(mre[:], mcol[:], 0.1)
            nc.vector.reciprocal(mre[:], mre[:])
            nc.vector.tensor_scalar_mul(mre[:], mre[:], float(dtf))
            force = Gp.tile([128, D], f32, tag="force")
            nc.vector.scalar_tensor_tensor(
                force[:], sb_pos[:, b], fm_b[:, 48:49], fm_b[:, 0:48],
                ALU.mult, ALU.subtract,
            )
            nc.vector.tensor_scalar_mul(force[:], force[:], mre[:])
            nv = Gp.tile([128, D], f32, tag="nv")
            nc.vector.scalar_tensor_tensor(
                nv[:], sb_vel[:, b], float(dampingf), force[:], ALU.mult, ALU.add
            )
            npos = Gp.tile([128, D], f32, tag="npos")
            nc.vector.scalar_tensor_tensor(
                npos[:], nv[:], float(dtf), sb_pos[:, b], ALU.mult, ALU.add
            )
            dma(out=out_pos[b], in_=npos[:])
            dma(out=out_vel[b], in_=nv[:])

    nc.compile()
    return nc


def _prep_core(c, pos, vel):
    jlo, jhi = NB * (c + 1), N - NB * c
    nlo = c + 1
    lo = np.arange(NB * c, NB * c + NB)
    hi = np.arange(N - NB * (c + 1), N - NB * c)
    rows = np.concatenate([lo, hi])
    catj = np.concatenate([np.arange(jlo), np.arange(jhi)])
    acti = np.concatenate([lo if jt < nlo else hi for jt in range(NJT)])

    pi = pos[:, acti]
    pj = pos[:, catj]
    one_b1 = np.ones((B, 1, CAT), np.float32)
    d = {
        "piT_gram": np.concatenate(
            [-2.0 * pi.transpose(0, 2, 1), one_b1,
             (pi * pi).sum(-1)[:, None, :]], axis=1),
        "piT_mlp": np.ascontiguousarray(pi.transpose(0, 2, 1)),
        "pjT": np.concatenate(
            [pj.transpose(0, 2, 1), (pj * pj).sum(-1)[:, None, :], one_b1],
            axis=1),
        "pj1": np.concatenate([pj, np.ones((B, CAT, 1), np.float32)], axis=2),
        "pos_nat": np.ascontiguousarray(pos[:, rows]),
        "vel_nat": np.ascontiguousarray(vel[:, rows]),
    }
    pm = np.zeros((NJT, NB, 128), np.float32)
    for jt in range(NJT):
        p0 = 0 if jt < nlo else 64
        pm[jt, np.arange(NB), p0 + np.arange(NB)] = 1.0
    d["perm"] = pm
    tri = (np.arange(NB)[None, :] < np.arange(NB)[:, None]).astype(np.float32)
    mk = np.ones((NJT, NB, NB), np.float32)
    mk[c] = tri          # lo-diagonal tile (jt == c < nlo always)
    mk[NJT - 1] = tri    # hi-diagonal tile (always the last)
    d["maskc"] = mk
    d = {k: np.ascontiguousarray(v, dtype=np.float32) for k, v in d.items()}
    return d, rows


def _shared_inputs(m_w1, m_b1, m_w2, f_w1, f_b1, f_w2, f_b2, f_w3):
    w1 = np.asarray(f_w1, np.float32)
    w2 = np.asarray(f_w2, np.float32)
    w3 = np.asarray(f_w3, np.float32).reshape(48)
    w1blk = np.zeros((6, 96), np.float32)
    w1blk[0:3, 0:48] = w1
    w1blk[3:6, 48:96] = w1
    w2blk = np.zeros((96, 96), np.float32)
    w2blk[0:48, 0:48] = w2
    w2blk[48:96, 48:96] = w2
    w3s = np.zeros((96, 4, 8), np.float32)
    for s in range(4):
        w3s[0:48, s, 2 * s] = w3
        w3s[48:96, s, 2 * s + 1] = w3
    b1 = np.asarray(f_b1, np.float32).reshape(48)
    b2 = np.asarray(f_b2, np.float32).reshape(48)
    return {
        "m_w1": np.ascontiguousarray(np.asarray(m_w1, np.float32)),
        "m_b1c": np.asarray(m_b1, np.float32).reshape(12, 1).copy(),
        "m_w2": np.ascontiguousarray(np.asarray(m_w2, np.float32)),
        "w1blk": w1blk,
        "w2blk": w2blk,
        "w3s": w3s,
        "b1blk": np.concatenate([b1, b1]).reshape(96, 1).copy(),
        "b2blk": np.concatenate([b2, b2]).reshape(96, 1).copy(),
        "ident64": np.eye(64, dtype=np.float32),
    }


def kernel(pos, vel, m_w1, m_b1, m_w2, m_b2, f_w1, f_b1, f_w2, f_b2, f_w3,
           f_b3, damping, dt):
    from concourse.bass_utils import run_bass_kernel_spmd

    pos = np.asarray(pos, np.float32)
    vel = np.asarray(vel, np.float32)
    key = (
        float(np.asarray(damping).ravel()[0]),
        float(np.asarray(dt).ravel()[0]),
        float(np.asarray(m_b2).ravel()[0]),
        float(np.asarray(f_b3).ravel()[0]),
    )
    if key not in _BUILD_CACHE:
        _BUILD_CACHE[key] = _build(*key)
    nc = _BUILD_CACHE[key]

    shared = _shared_inputs(m_w1, m_b1, m_w2, f_w1, f_b1, f_w2, f_b2, f_w3)
    in_maps, rows_l = [], []
    for c in range(NCORES):
        d, rows = _prep_core(c, pos, vel)
        d.update(shared)
        in_maps.append(d)
        rows_l.append(rows)

    res = run_bass_kernel_spmd(nc, in_maps, core_ids=list(range(NCORES)))
    new_pos = np.empty((B, N, D), np.float32)
    new_vel = np.empty((B, N, D), np.float32)
    for c in range(NCORES):
        new_pos[:, rows_l[c]] = res.results[c]["out_pos"]
        new_vel[:, rows_l[c]] = res.results[c]["out_vel"]
    return new_pos, new_vel


# revision 12
# speedup vs baseline: 1.1667x; 1.1667x over previous
"""Trainium2 Bass kernel for nn_GravityLayer (gnn_message_passing).

Sharding: causal fold — core c owns rows [64c,64c+64) ("lo") and
[1024-64(c+1),1024-64c) ("hi"). Per-core j-work is uniform: Jlo+Jhi = 1088
= 17 j-tiles of 64. All per-core variation is baked into host-prepared
input *content* (cat-j gathers, masks, one-hot placement), so the SPMD
program is identical on all 8 cores.

Device pipeline per (batch, j-tile): dist via K=50 Gram matmul
(-2 pos_i.pos_j + ri + rj), clamp+sqrt(ACT)+recip*mask(DVE); pair-MLP as
block-diagonal 2-pair fp32r matmuls [6,96]->gelu->[96,96]->gelu->[96,8]
(4 shifted W3 variants accumulate into one [8,512] PSUM bank); reshape to
G[64i,64j] (2 DMAs); G=(G+b3)*R; PE-transpose; contraction with
[pos_j|1] accumulates [64,49]; one-hot perm matmul places into the
fold-row frame [128, 49] PSUM accumulator. Finals elementwise on DVE.
"""

import sys
import numpy as np

for _p in ("/opt/trn_rl_repo",):
    if _p not in sys.path:
        sys.path.insert(0, _p)

B, N, D = 2, 1024, 48
NCORES = 8
NB = 64           # fold block / j-tile size
NJT = 17          # j-tiles per batch per core
CAT = NJT * NB    # 1088
EPS2 = 1e-4       # 0.01**2

_BUILD_CACHE = {}


def _build(dampingf, dtf, mb2f, fb3f):
    import concourse.bass as bass
    import concourse.bacc as bacc
    import concourse.tile as tile
    from concourse import mybir
    from contextlib import ExitStack

    f32 = mybir.dt.float32
    f32r = mybir.dt.float32r
    AF = mybir.ActivationFunctionType
    ALU = mybir.AluOpType
    AP = bass.AP

    nc = bacc.Bacc("TRN2")

    def inp(name, shape, dt_=None):
        return nc.declare_dram_parameter(name, list(shape), dt_ or f32,
                                         isOutput=False)

    piT_gram = inp("piT_gram", (B, 50, CAT))
    piT_mlp = inp("piT_mlp", (B, 48, CAT))
    pjT = inp("pjT", (B, 50, CAT))
    pj1 = inp("pj1", (B, CAT, 49))
    perm = inp("perm", (NJT, NB, 128))
    maskc = inp("maskc", (NJT, NB, NB))
    pos_nat = inp("pos_nat", (B, 128, D))
    vel_nat = inp("vel_nat", (B, 128, D))
    m_w1 = inp("m_w1", (48, 12))
    m_b1c = inp("m_b1c", (12, 1))
    m_w2 = inp("m_w2", (12, 1))
    w1blk_i = inp("w1blk", (6, 96), f32r)
    w2blk_i = inp("w2blk", (96, 96), f32r)
    w3s_i = inp("w3s", (96, 4, 8), f32r)
    b1blk_i = inp("b1blk", (96, 1))
    b2blk_i = inp("b2blk", (96, 1))
    ident_i = inp("ident64", (64, 64))

    out_pos = nc.declare_dram_parameter("out_pos", [B, 128, D], f32, isOutput=True)
    out_vel = nc.declare_dram_parameter("out_vel", [B, 128, D], f32, isOutput=True)

    with ExitStack() as ctx:
        tc = ctx.enter_context(tile.TileContext(nc))
        S = ctx.enter_context(tc.tile_pool(name="S", bufs=1))
        Xp = ctx.enter_context(tc.tile_pool(name="Xp", bufs=3))
        Hp = ctx.enter_context(tc.tile_pool(name="Hp", bufs=2))
        Gp = ctx.enter_context(tc.tile_pool(name="Gp", bufs=2))
        PZ = ctx.enter_context(tc.tile_pool(name="PZ", bufs=1, space="PSUM"))
        PZ2 = ctx.enter_context(tc.tile_pool(name="PZ2", bufs=1, space="PSUM"))
        PF = ctx.enter_context(tc.tile_pool(name="PF", bufs=1, space="PSUM"))
        PG = ctx.enter_context(tc.tile_pool(name="PG", bufs=1, space="PSUM"))
        PM = ctx.enter_context(tc.tile_pool(name="PM", bufs=1, space="PSUM"))

        dma = nc.sync.dma_start

        # ---- resident SBUF ----
        sb_pjT = S.tile([50, B, CAT], f32)
        dma(out=sb_pjT[:], in_=pjT.rearrange("b r c -> r b c"))
        sb_pig = S.tile([50, B, CAT], f32)
        dma(out=sb_pig[:], in_=piT_gram.rearrange("b r c -> r b c"))
        sb_pim = S.tile([48, B, CAT], f32)
        dma(out=sb_pim[:], in_=piT_mlp.rearrange("b r c -> r b c"))
        sb_pj1 = S.tile([NB, B, NJT, 49], f32)
        for b in range(B):
            dma(out=sb_pj1[:, b], in_=pj1[b].rearrange("(t j) c -> j t c", j=NB))
        sb_perm = S.tile([NB, NJT, 128], f32)
        dma(out=sb_perm[:], in_=perm.rearrange("t j c -> j t c"))
        sb_mask = S.tile([NB, NJT, NB], f32)
        dma(out=sb_mask[:], in_=maskc.rearrange("t j c -> j t c"))
        sb_pos = S.tile([128, B, D], f32)
        dma(out=sb_pos[:], in_=pos_nat.rearrange("b i c -> i b c"))
        sb_vel = S.tile([128, B, D], f32)
        dma(out=sb_vel[:], in_=vel_nat.rearrange("b i c -> i b c"))
        sb_mw1 = S.tile([48, 12], f32)
        dma(out=sb_mw1[:], in_=m_w1[:])
        sb_mb1 = S.tile([12, 1], f32)
        dma(out=sb_mb1[:], in_=m_b1c[:])
        sb_mw2 = S.tile([12, 1], f32)
        dma(out=sb_mw2[:], in_=m_w2[:])
        w1blk = S.tile([6, 96], f32r)
        dma(out=w1blk[:], in_=w1blk_i[:])
        w2blk = S.tile([96, 96], f32r)
        dma(out=w2blk[:], in_=w2blk_i[:])
        w3s = S.tile([96, 4, 8], f32r)
        dma(out=w3s[:], in_=w3s_i[:])
        b1blk = S.tile([96, 1], f32)
        dma(out=b1blk[:], in_=b1blk_i[:])
        b2blk = S.tile([96, 1], f32)
        dma(out=b2blk[:], in_=b2blk_i[:])
        ident = S.tile([64, 64], f32)
        dma(out=ident[:], in_=ident_i[:])

        sb_mb2 = S.tile([1, 1], f32)
        nc.vector.memset(sb_mb2[:], float(mb2f))
        ones11 = S.tile([1, 1], f32)
        nc.vector.memset(ones11[:], 1.0)
        ones164 = S.tile([1, 64], f32)
        nc.vector.memset(ones164[:], 1.0)
        zeros64 = S.tile([64, 64], f32)
        nc.vector.memset(zeros64[:], 0.0)

        D_all = S.tile([NB, B, NJT, NB], f32)
        R_all = S.tile([NB, B, NJT, NB], f32)
        MIB_all = S.tile([NB, B, NJT, NB], f32)
        MJB_all = S.tile([NB, B, NJT, NB], f32)
        gT_all = S.tile([NB, B, NJT, NB], f32)
        m_i = S.tile([1, B, CAT], f32)
        m_j = S.tile([1, B, CAT], f32)

        SEGS = [(0, 512), (512, 512), (1024, 64)]

        # ---- phase B: masses (gelu, then softplus) ----
        mh_tiles = {}
        for b in range(B):
            for tag, src in (("i", sb_pim[:, b]), ("j", sb_pjT[0:48, b])):
                mh = Hp.tile([12, CAT], f32, tag="mh" + tag + str(b))
                for o, w in SEGS:
                    pm1 = PZ.tile([12, 512], f32, tag="z1")
                    nc.tensor.matmul(
                        pm1[:, 0:w],
                        sb_mw1[:],
                        src[:, o : o + w],
                    )
                    nc.scalar.activation(
                        mh[:, o : o + w], pm1[:, 0:w], AF.Gelu, bias=sb_mb1[:]
                    )
                mh_tiles[(b, tag)] = mh
        tc.no_sync_barrier()
        # softplus(x) = ln(1 + exp(x)); batch all Exp, then all Ln
        sp_all = S.tile([1, 4, CAT], f32)
        k = 0
        for b in range(B):
            for tag in ("i", "j"):
                mh = mh_tiles[(b, tag)]
                for o, w in SEGS:
                    pm2 = PF.tile([1, 512], f32, tag="F")
                    nc.tensor.matmul(
                        pm2[:, 0:w],
                        sb_mw2[:],
                        mh[:, o : o + w],
                    )
                    nc.scalar.activation(
                        sp_all[:, k, o : o + w], pm2[:, 0:w], AF.Exp,
                        bias=sb_mb2[:],
                    )
                k += 1
        tc.no_sync_barrier()
        k = 0
        for b in range(B):
            for tag, dst in (("i", m_i[:, b]), ("j", m_j[:, b])):
                nc.scalar.activation(dst, sp_all[:, k], AF.Ln, bias=1.0)
                k += 1

        tc.no_sync_barrier()
        # ---- phase A: distances ----
        for b in range(B):
            for jt in range(NJT):
                c0 = jt * NB
                pd2 = PG.tile([64, 64], f32, tag="gtp")
                nc.tensor.matmul(
                    pd2[:],
                    sb_pig[:, b, c0 : c0 + NB],
                    sb_pjT[:, b, c0 : c0 + NB],
                )
                nc.vector.tensor_scalar_max(pd2[:], pd2[:], EPS2)
                nc.scalar.activation(D_all[:, b, jt], pd2[:], AF.Sqrt)
                nc.vector.reciprocal(R_all[:, b, jt], D_all[:, b, jt])
                nc.vector.tensor_mul(R_all[:, b, jt], R_all[:, b, jt], sb_mask[:, jt])
                micol = PG.tile([64, 1], f32, tag="fm")
                nc.tensor.matmul(micol[:], m_i[:, b, c0 : c0 + NB], ones11[:])
                nc.vector.tensor_scalar_add(
                    MIB_all[:, b, jt], zeros64[:], micol[:]
                )
                mjb = PG.tile([64, 64], f32, tag="gtp")
                nc.tensor.matmul(mjb[:], ones164[:], m_j[:, b, c0 : c0 + NB])
                nc.vector.tensor_copy(MJB_all[:, b, jt], mjb[:])

        tc.no_sync_barrier()
        # ---- phase C: force MLP ----
        FM = PM.tile([128, B * 49], f32)
        for b in range(B):
            for jt in range(NJT):
                c0 = jt * NB
                X = Xp.tile([6, 2048], f32r, tag="X")
                xap = X[:]
                # rows {0,3}: dist, i-major flatten of the [64,64] D tile
                nc.gpsimd.dma_start(
                    out=AP(tensor=xap.tensor, offset=xap.offset,
                           ap=[[3 * 2048, 2], [1, 2048]]),
                    in_=D_all[:, b, jt].bitcast(f32r),
                )
                # rows {1,4}: m_i broadcast tile, i-major flatten
                nc.gpsimd.dma_start(
                    out=AP(tensor=xap.tensor, offset=xap.offset + 2048,
                           ap=[[3 * 2048, 2], [1, 2048]]),
                    in_=MIB_all[:, b, jt].bitcast(f32r),
                )
                # rows {2,5}: m_j broadcast tile
                nc.gpsimd.dma_start(
                    out=AP(tensor=xap.tensor, offset=xap.offset + 2 * 2048,
                           ap=[[3 * 2048, 2], [1, 2048]]),
                    in_=MJB_all[:, b, jt].bitcast(f32r),
                )

                F = PF.tile([8, 512], f32, tag="F")
                for p in range(2):
                    z1 = PZ.tile([96, 1024], f32, tag="z1")
                    for q in range(2):
                        o = 512 * q
                        nc.tensor.matmul(
                            z1[:, o : o + 512],
                            w1blk[:],
                            X[:, 1024 * p + o : 1024 * p + o + 512],
                        )
                    h1 = Hp.tile([96, 1024], f32r, tag="h1")
                    nc.scalar.activation(h1[:], z1[:], AF.Gelu, bias=b1blk[:])
                    z2 = PZ2.tile([96, 1024], f32, tag="z2")
                    for q in range(2):
                        o = 512 * q
                        nc.tensor.matmul(
                            z2[:, o : o + 512],
                            w2blk[:],
                            h1[:, o : o + 512],
                        )
                    h2 = Hp.tile([96, 1024], f32r, tag="h2")
                    nc.scalar.activation(h2[:], z2[:], AF.Gelu, bias=b2blk[:])
                    for q in range(2):
                        s = 2 * p + q
                        nc.tensor.matmul(
                            F[:],
                            w3s[:, s],
                            h2[:, 512 * q : 512 * q + 512],
                            start=(s == 0),
                            stop=(s == 3),
                        )
                f8 = Gp.tile([8, 512], f32, tag="f8")
                nc.vector.tensor_copy(f8[:], F[:])
                G = Gp.tile([64, 64], f32, tag="G")
                f8ap = f8[:]
                for a in range(2):
                    dma(
                        out=G[32 * a : 32 * a + 32, :],
                        in_=AP(tensor=f8ap.tensor, offset=f8ap.offset + a * 512,
                               ap=[[2 * 512, 4], [1, 512]]),
                    )
                nc.vector.scalar_tensor_tensor(
                    G[:], G[:], fb3f, R_all[:, b, jt], ALU.add, ALU.mult
                )
                gT = gT_all[:, b, jt]
                for br in range(2):
                    for bc in range(2):
                        nc.vector.transpose(
                            gT[32 * bc : 32 * bc + 32, 32 * br : 32 * br + 32],
                            G[32 * br : 32 * br + 32, 32 * bc : 32 * bc + 32],
                        )

        # ---- phase D: contraction + placement (dense PE tail) ----
        for b in range(B):
            for jt in range(NJT):
                fm = PG.tile([64, 49], f32, tag="fm")
                nc.tensor.matmul(fm[:], gT_all[:, b, jt], sb_pj1[:, b, jt])
                fm_sb = Gp.tile([64, 49], f32, tag="fmsb")
                nc.vector.tensor_copy(fm_sb[:], fm[:])
                nc.tensor.matmul(
                    FM[:, 49 * b : 49 * b + 49],
                    sb_perm[:, jt],
                    fm_sb[:],
                    start=(jt == 0),
                    stop=(jt == NJT - 1),
                )

        # ---- finals ----
        for b in range(B):
            fm_b = FM[:, 49 * b : 49 * b + 49]
            mcol = PG.tile([128, 1], f32, tag="fm")
            nc.tensor.matmul(mcol[0:64], m_i[:, b, 0:64], ones11[:])
            nc.tensor.matmul(
                mcol[64:128], m_i[:, b, 16 * NB : 16 * NB + 64], ones11[:]
            )
            mre = Gp.tile([128, 1], f32, tag="mre")
            nc.vector.tensor_scalar_add(mre[:], mcol[:], 0.1)
            nc.vector.reciprocal(mre[:], mre[:])
            nc.vector.tensor_scalar_mul(mre[:], mre[:], float(dtf))
            force = Gp.tile([128, D], f32, tag="force")
            nc.vector.scalar_tensor_tensor(
                force[:], sb_pos[:, b], fm_b[:, 48:49], fm_b[:, 0:48],
                ALU.mult, ALU.subtract,
            )
            nc.vector.tensor_scalar_mul(force[:], force[:], mre[:])
            nv = Gp.tile([128, D], f32, tag="nv")
            nc.vector.scalar_tensor_tensor(
                nv[:], sb_vel[:, b], float(dampingf), force[:], ALU.mult, ALU.add
            )
            npos = Gp.tile([128, D], f32, tag="npos")
            nc.vector.scalar_tensor_tensor(
                npos[:], nv[:], float(dtf), sb_pos[:, b], ALU.mult, ALU.add
            )
            dma(out=out_pos[b], in_=npos[:])
            dma(out=out_vel[b], in_=nv[:])

    nc.compile()
    return nc


def _prep_core(c, pos, vel):
    jlo, jhi = NB * (c + 1), N - NB * c
    nlo = c + 1
    lo = np.arange(NB * c, NB * c + NB)
    hi = np.arange(N - NB * (c + 1), N - NB * c)
    rows = np.concatenate([lo, hi])
    catj = np.concatenate([np.arange(jlo), np.arange(jhi)])
    acti = np.concatenate([lo if jt < nlo else hi for jt in range(NJT)])

    pi = pos[:, acti]
    pj = pos[:, catj]
    one_b1 = np.ones((B, 1, CAT), np.float32)
    d = {
        "piT_gram": np.concatenate(
            [-2.0 * pi.transpose(0, 2, 1), one_b1,
             (pi * pi).sum(-1)[:, None, :]], axis=1),
        "piT_mlp": np.ascontiguousarray(pi.transpose(0, 2, 1)),
        "pjT": np.concatenate(
            [pj.transpose(0, 2, 1), (pj * pj).sum(-1)[:, None, :], one_b1],
            axis=1),
        "pj1": np.concatenate([pj, np.ones((B, CAT, 1), np.float32)], axis=2),
        "pos_nat": np.ascontiguousarray(pos[:, rows]),
        "vel_nat": np.ascontiguousarray(vel[:, rows]),
    }
    pm = np.zeros((NJT, NB, 128), np.float32)
    for jt in range(NJT):
        p0 = 0 if jt < nlo else 64
        pm[jt, np.arange(NB), p0 + np.arange(NB)] = 1.0
    d["perm"] = pm
    tri = (np.arange(NB)[None, :] < np.arange(NB)[:, None]).astype(np.float32)
    mk = np.ones((NJT, NB, NB), np.float32)
    mk[c] = tri          # lo-diagonal tile (jt == c < nlo always)
    mk[NJT - 1] = tri    # hi-diagonal tile (always the last)
    d["maskc"] = mk
    d = {k: np.ascontiguousarray(v, dtype=np.float32) for k, v in d.items()}
    return d, rows


def _shared_inputs(m_w1, m_b1, m_w2, f_w1, f_b1, f_w2, f_b2, f_w3):
    w1 = np.asarray(f_w1, np.float32)
    w2 = np.asarray(f_w2, np.float32)
    w3 = np.asarray(f_w3, np.float32).reshape(48)
    w1blk = np.zeros((6, 96), np.float32)
    w1blk[0:3, 0:48] = w1
    w1blk[3:6, 48:96] = w1
    w2blk = np.zeros((96, 96), np.float32)
    w2blk[0:48, 0:48] = w2
    w2blk[48:96, 48:96] = w2
    w3s = np.zeros((96, 4, 8), np.float32)
    for s in range(4):
        w3s[0:48, s, 2 * s] = w3
        w3s[48:96, s, 2 * s + 1] = w3
    b1 = np.asarray(f_b1, np.float32).reshape(48)
    b2 = np.asarray(f_b2, np.float32).reshape(48)
    return {
        "m_w1": np.ascontiguousarray(np.asarray(m_w1, np.float32)),
        "m_b1c": np.asarray(m_b1, np.float32).reshape(12, 1).copy(),
        "m_w2": np.ascontiguousarray(np.asarray(m_w2, np.float32)),
        "w1blk": w1blk,
        "w2blk": w2blk,
        "w3s": w3s,
        "b1blk": np.concatenate([b1, b1]).reshape(96, 1).copy(),
        "b2blk": np.concatenate([b2, b2]).reshape(96, 1).copy(),
        "ident64": np.eye(64, dtype=np.float32),
    }


def kernel(pos, vel, m_w1, m_b1, m_w2, m_b2, f_w1, f_b1, f_w2, f_b2, f_w3,
           f_b3, damping, dt):
    from concourse.bass_utils import run_bass_kernel_spmd

    pos = np.asarray(pos, np.float32)
    vel = np.asarray(vel, np.float32)
    key = (
        float(np.asarray(damping).ravel()[0]),
        float(np.asarray(dt).ravel()[0]),
        float(np.asarray(m_b2).ravel()[0]),
        float(np.asarray(f_b3).ravel()[0]),
    )
    if key not in _BUILD_CACHE:
        _BUILD_CACHE[key] = _build(*key)
    nc = _BUILD_CACHE[key]

    shared = _shared_inputs(m_w1, m_b1, m_w2, f_w1, f_b1, f_w2, f_b2, f_w3)
    in_maps, rows_l = [], []
    for c in range(NCORES):
        d, rows = _prep_core(c, pos, vel)
        d.update(shared)
        in_maps.append(d)
        rows_l.append(rows)

    res = run_bass_kernel_spmd(nc, in_maps, core_ids=list(range(NCORES)))
    new_pos = np.empty((B, N, D), np.float32)
    new_vel = np.empty((B, N, D), np.float32)
    for c in range(NCORES):
        new_pos[:, rows_l[c]] = res.results[c]["out_pos"]
        new_vel[:, rows_l[c]] = res.results[c]["out_vel"]
    return new_pos, new_vel
